# revision 29
# baseline (speedup 1.0000x reference)
import numpy as np
import ml_dtypes

BF = ml_dtypes.bfloat16
B, N, WT, F, H = 64, 512, 24, 16, 128
NL = N // 8   # nodes per core
NT = WT // 2  # timestep pairs
KC = 49       # fused input contraction: 3 hops * 16 f + ones row

# xstack partition layout (rows of the K=49 contraction):
#   p 0-15  : hop0 (raw x features)
#   p 16    : ones (carries all input-side biases)
#   p 17-32 : hop1 (A @ x features)
#   p 33-48 : hop2 (A^2 @ x features)


def _build(nc, bass, mybir, tile):
    f32 = mybir.dt.float32
    bf16 = mybir.dt.bfloat16
    Alu = mybir.AluOpType
    Act = mybir.ActivationFunctionType

    t_a12 = nc.dram_tensor("a12", [128, 4, 2, NL], bf16, kind="ExternalInput").ap()
    t_xm = nc.dram_tensor("xm", [128, B, 4, WT * F], bf16, kind="ExternalInput").ap()
    # hop0 features + ones row, per timestep pair: [pair, 17, 2, nl, b]
    t_x0b = nc.dram_tensor("x0b", [NT, 17, 2, NL, B], bf16, kind="ExternalInput").ap()
    t_wc = nc.dram_tensor("wc", [128, NL, 3, 128], bf16, kind="ExternalInput").ap()
    t_whh = nc.dram_tensor("whh", [128, NL, 3, 128], bf16, kind="ExternalInput").ap()
    t_bhn = nc.dram_tensor("bhn", [128, 8, 128], bf16, kind="ExternalInput").ap()
    t_ind = nc.dram_tensor("ind", [128, 8, B], bf16, kind="ExternalInput").ap()
    t_zpad = nc.dram_tensor("zpad", [128 - KC, 2, NL, B], bf16,
                            kind="ExternalInput").ap()
    t_wout = nc.dram_tensor("wout", [128, F], bf16, kind="ExternalInput").ap()
    t_bout = nc.dram_tensor("bout", [128, F], f32, kind="ExternalInput").ap()
    t_out = nc.dram_tensor("out", [128, 32, F], f32, kind="ExternalOutput").ap()

    with tile.TileContext(nc) as tc:
        with (
            tc.tile_pool(name="const", bufs=1) as cpool,
            tc.tile_pool(name="hpool", bufs=1) as hpool,
            tc.tile_pool(name="stage", bufs=1) as spool,
        ):
            a12 = cpool.tile([128, 4, 2, NL], bf16)
            wc = cpool.tile([128, NL, 3, 128], bf16)
            whh = cpool.tile([128, NL, 3, 128], bf16)
            bhn = cpool.tile([128, 8, 128], bf16)
            ind = cpool.tile([128, 8, B], bf16)
            wout = cpool.tile([128, F], bf16)
            bout = cpool.tile([128, F], f32)
            for sb, dr in [(a12, t_a12), (wc, t_wc), (whh, t_whh),
                           (bhn, t_bhn), (ind, t_ind), (wout, t_wout),
                           (bout, t_bout)]:
                nc.sync.dma_start(sb[:], dr[:])
            h = hpool.tile([128, NL, B], bf16)
            nc.any.memset(h[:], 0.0)

            # staging for diffusion hops, baseline layout [(w%8)*16+f, cc, nl, b]
            xc1 = spool.tile([128, 3, NL, B], bf16)
            xc2 = spool.tile([128, 3, NL, B], bf16)

            # ---- phase 1: diffusion hops (contract over source nodes) ----
            # 4 batch elems per PSUM tile; one matmul covers both hops
            with (
                tc.tile_pool(name="xmb", bufs=2) as xmpool,
                tc.tile_pool(name="p1", bufs=2, space="PSUM") as p1pool,
            ):
                for bq in range(B // 4):
                    xmb = xmpool.tile([128, 4, 4, WT * F], bf16)
                    nc.sync.dma_start(xmb[:], t_xm[:, 4 * bq:4 * bq + 4])
                    P12 = p1pool.tile([128, 3, 2, NL, 4], f32, tag="P12")
                    for bs in range(4):
                        for mc in range(4):
                            for cc in range(3):
                                lhsT = xmb[:, bs, mc, 128 * cc:128 * cc + 128]
                                nc.tensor.matmul(P12[:, cc, :, :, bs], lhsT,
                                                 a12[:, mc, :, :],
                                                 start=(mc == 0), stop=(mc == 3))
                    nc.vector.tensor_copy(xc1[:, :, :, 4 * bq:4 * bq + 4],
                                          P12[:, :, 0, :, :])
                    nc.scalar.copy(xc2[:, :, :, 4 * bq:4 * bq + 4],
                                   P12[:, :, 1, :, :])

            # ---- phase 2: GRU over time, input projection fused into Wc ----
            with (
                tc.tile_pool(name="xs", bufs=2) as xspool,
                tc.tile_pool(name="gp", bufs=2, space="PSUM") as gppool,
                tc.tile_pool(name="pn", bufs=2, space="PSUM") as pnpool,
                tc.tile_pool(name="hn", bufs=2, space="PSUM") as hnpool,
                tc.tile_pool(name="ew", bufs=2) as ewpool,
            ):
                for t in range(NT):
                    xs = xspool.tile([128, 2, NL, B], bf16, tag="xs")
                    if t < 2:
                        # rows 49-127 are contracted against zero weight rows;
                        # zero them once per ring buffer so no NaNs flow
                        nc.sync.dma_start(xs[KC:128, :, :, :], t_zpad[:])
                    # hop0 rows 0-15 + ones row 16 straight from DRAM
                    nc.sync.dma_start(xs[0:17, :, :, :], t_x0b[t])
                    # hop1/hop2 rows via SBUF->SBUF partition restructure
                    for wsub in range(2):
                        w = 2 * t + wsub
                        wo, ccw = w % 8, w // 8
                        src1 = xc1[16 * wo:16 * wo + 16, ccw, :, :]
                        src2 = xc2[16 * wo:16 * wo + 16, ccw, :, :]
                        nc.sync.dma_start(xs[17:33, wsub, :, :], src1)
                        nc.sync.dma_start(xs[33:49, wsub, :, :], src2)

                    for wsub in range(2):
                        for g in range(8):
                            ns = slice(8 * g, 8 * g + 8)
                            Prz = gppool.tile([128, 8, 2, B], f32, tag="Prz")
                            Pn = pnpool.tile([128, 8, B], f32, tag="Pn")
                            Phn = hnpool.tile([128, 8, B], f32, tag="Phn")
                            # b_hn via indicator matmul (opens accumulation)
                            nc.tensor.matmul(Phn[:], bhn[:, g, :], ind[:],
                                             start=True, stop=False,
                                             skip_group_check=True)
                            for j in range(8):
                                n = 8 * g + j
                                xsn = xs[:, wsub, n, :]
                                hn_ = h[:, n, :]
                                for gc in range(2):
                                    o = Prz[:, j, gc, :]
                                    nc.tensor.matmul(o, wc[:, n, gc, :], xsn,
                                                     start=True, stop=False)
                                    nc.tensor.matmul(o, whh[:, n, gc, :], hn_,
                                                     start=False, stop=True)
                                nc.tensor.matmul(Pn[:, j, :], wc[:, n, 2, :],
                                                 xsn, start=True, stop=True)
                                nc.tensor.matmul(Phn[:, j, :], whh[:, n, 2, :],
                                                 hn_, start=False,
                                                 stop=(j == 7),
                                                 skip_group_check=True)
                            # ---- gate elementwise ----
                            rz = ewpool.tile([128, 8, 2, B], bf16, tag="rz")
                            nc.scalar.activation(rz[:], Prz[:], Act.Sigmoid)
                            tt = ewpool.tile([128, 8, B], bf16, tag="tt")
                            nc.vector.tensor_tensor(tt[:], rz[:, :, 0, :],
                                                    Phn[:], Alu.mult)
                            ut = ewpool.tile([128, 8, B], bf16, tag="ut")
                            nc.vector.tensor_tensor(ut[:], tt[:], Pn[:],
                                                    Alu.add)
                            nt = ewpool.tile([128, 8, B], bf16, tag="nt")
                            nc.scalar.activation(nt[:], ut[:], Act.Tanh)
                            st = ewpool.tile([128, 8, B], bf16, tag="st")
                            nc.gpsimd.tensor_tensor(st[:], h[:, ns, :], nt[:],
                                                    Alu.subtract)
                            vt = ewpool.tile([128, 8, B], bf16, tag="vt")
                            nc.vector.tensor_tensor(vt[:], rz[:, :, 1, :], st[:],
                                                    Alu.mult)
                            nc.vector.tensor_tensor(h[:, ns, :], nt[:], vt[:],
                                                    Alu.add)

            # ---- output projection ----
            with (
                tc.tile_pool(name="po", bufs=1, space="PSUM") as popool,
                tc.tile_pool(name="ou", bufs=1) as oupool,
            ):
                Po = popool.tile([128, 32, F], f32)
                for c in range(32):
                    nc.tensor.matmul(Po[:, c, :], h[:, 2 * c:2 * c + 2, :],
                                     wout[:], start=True, stop=True)
                outsb = oupool.tile([128, 32, F], f32)
                nc.vector.tensor_tensor(
                    outsb[:], Po[:], bout[:, None, :].to_broadcast((128, 32, F)),
                    Alu.add)
                nc.sync.dma_start(t_out[:], outsb[:])
    nc.compile()


def kernel(**inputs):
    import concourse.bacc as bacc
    import concourse.bass as bass
    import concourse.mybir as mybir
    import concourse.tile as tile
    from concourse import bass_utils

    x = np.asarray(inputs["x"], np.float32)
    A = np.asarray(inputs["A_fw"], np.float32)
    dcw = np.asarray(inputs["dc_weights"], np.float32)
    W_ih = np.asarray(inputs["W_ih"], np.float32)
    W_hh = np.asarray(inputs["W_hh"], np.float32)
    b_ih = np.asarray(inputs["b_ih"], np.float32)
    b_hh = np.asarray(inputs["b_hh"], np.float32)
    W_out = np.asarray(inputs["W_out"], np.float32)
    b_out = np.asarray(inputs["b_out"], np.float32)

    A2 = A @ A
    dc_all = np.stack([dcw[0:16], dcw[16:32] + dcw[32:48], dcw[48:64] + dcw[64:80]])
    xbf = x.astype(BF)
    xm = np.ascontiguousarray(xbf.reshape(B, 4, 128, WT * F).transpose(2, 0, 1, 3))
    wout_h = W_out.astype(BF)
    bout_h = np.tile(b_out[None, :], (128, 1)).astype(np.float32)

    # fused input weights: Wc[n, gate, j, (hop, f)] = sum_h W_ih[n, gj, h] dc[hop, f, h]
    Wg = W_ih.reshape(N, 3, 128, H)
    wc_full = np.einsum('ngjh,ofh->ngjof', Wg, dc_all)        # [N, 3, 128, 3, 16]
    bias_in = b_ih.reshape(N, 3, 128).copy()
    bias_in[:, 0:2, :] += b_hh.reshape(N, 3, 128)[:, 0:2, :]  # r,z combined bias
    # K-layout: rows 0-15 hop0, 16 ones(bias), 17-32 hop1, 33-48 hop2,
    # 49-127 zero padding (keeps K=128 so fast weight load stays enabled)
    wc_k = np.zeros((N, 3, 128, 128), np.float32)
    wc_k[:, :, 0:16, :] = wc_full[:, :, :, 0, :].transpose(0, 1, 3, 2)
    wc_k[:, :, 16, :] = bias_in
    wc_k[:, :, 17:33, :] = wc_full[:, :, :, 1, :].transpose(0, 1, 3, 2)
    wc_k[:, :, 33:49, :] = wc_full[:, :, :, 2, :].transpose(0, 1, 3, 2)

    ind_h = np.zeros((128, 8, B), np.float32)
    for k in range(8):
        ind_h[k, k, :] = 1.0
    ind_h = ind_h.astype(BF)

    in_maps = []
    for c in range(8):
        ns = slice(c * NL, (c + 1) * NL)
        a1t = A[ns].T.astype(BF).reshape(4, 128, NL).transpose(1, 0, 2)
        a12 = np.ascontiguousarray(np.stack([a1t, A2[ns].T.astype(BF)
                                             .reshape(4, 128, NL)
                                             .transpose(1, 0, 2)], axis=2))
        xl = xbf[:, ns]  # [b, nl, w, f]
        # x0b[t, p, wsub, nl, b]: p 0-15 = f rows of x[w=2t+wsub], p16 = ones
        x0b = np.empty((NT, 17, 2, NL, B), np.float32)
        xw = np.asarray(xl, np.float32).transpose(2, 3, 1, 0)  # [w, f, nl, b]
        x0b[:, 0:16] = xw.reshape(NT, 2, 16, NL, B).transpose(0, 2, 1, 3, 4)
        x0b[:, 16] = 1.0
        wc_h = np.ascontiguousarray(
            wc_k[ns].transpose(2, 0, 1, 3)).astype(BF)       # [128, NL, 3, 128]
        whh_h = np.ascontiguousarray(
            W_hh[ns].transpose(2, 0, 1).astype(BF).reshape(128, NL, 3, 128))
        bhn_h = np.zeros((128, 8, 128), np.float32)           # [n8(+pad), g, j]
        bhn_h[0:8] = b_hh[ns, 256:384].reshape(8, 8, 128).transpose(1, 0, 2)
        bhn_h = bhn_h.astype(BF)
        in_maps.append({
            "a12": a12, "xm": xm,
            "x0b": np.ascontiguousarray(x0b).astype(BF),
            "wc": wc_h, "whh": whh_h, "bhn": bhn_h, "ind": ind_h,
            "zpad": np.zeros((128 - KC, 2, NL, B), BF),
            "wout": wout_h, "bout": bout_h,
        })

    nc = bacc.Bacc("TRN2", target_bir_lowering=False, debug=False, num_devices=8)
    _build(nc, bass, mybir, tile)
    import os, time
    trace = bool(os.environ.get("DGCN_TRACE"))
    res = bass_utils.run_bass_kernel_spmd(nc, in_maps, core_ids=list(range(8)),
                                          trace=trace)
    if trace and res.exec_time_ns:
        print(f"MEASURED exec_time_ns: {res.exec_time_ns}", flush=True)
        try:
            with open("/tmp/dgcn_exec_ns.txt", "w") as f:
                f.write(str(res.exec_time_ns))
        except Exception:
            pass
        if res.instructions_and_trace:
            print(f"trace: {res.instructions_and_trace[1]}", flush=True)
    if os.environ.get("DGCN_BENCH"):
        for it in range(int(os.environ["DGCN_BENCH"])):
            t0 = time.time()
            res = bass_utils.run_bass_kernel_spmd(nc, in_maps, core_ids=list(range(8)))
            print(f"bench iter {it}: {(time.time()-t0)*1e3:.1f} ms", flush=True)

    out = np.zeros((B, N, F), np.float32)
    for c in range(8):
        arr = res.results[c]["out"]  # [128, 32, F]
        tmp = arr.transpose(1, 0, 2).reshape(32, 2, B, F).transpose(2, 0, 1, 3)
        out[:, c * NL:(c + 1) * NL] = tmp.reshape(B, NL, F)
    return out


# revision 33
# speedup vs baseline: 1.1389x; 1.1389x over previous
import numpy as np
import ml_dtypes

BF = ml_dtypes.bfloat16
B, N, WT, F, H = 64, 512, 24, 16, 128
NL = N // 8   # nodes per core
NT = WT // 2  # timestep pairs
KC = 49       # fused input contraction: 3 hops * 16 f + ones row

# xstack partition layout (rows of the K=49 contraction):
#   p 0-15  : hop0 (raw x features)
#   p 16    : ones (carries all input-side biases)
#   p 17-32 : hop1 (A @ x features)
#   p 33-48 : hop2 (A^2 @ x features)


def _build(nc, bass, mybir, tile):
    f32 = mybir.dt.float32
    bf16 = mybir.dt.bfloat16
    Alu = mybir.AluOpType
    Act = mybir.ActivationFunctionType

    t_a12 = nc.dram_tensor("a12", [128, 4, 2, NL], bf16, kind="ExternalInput").ap()
    t_xm = nc.dram_tensor("xm", [3, 128, B, 4, 128], bf16, kind="ExternalInput").ap()
    # hop0 features + ones row, per timestep pair: [pair, 17, 2, nl, b]
    t_x0b = nc.dram_tensor("x0b", [NT, 17, 2, NL, B], bf16, kind="ExternalInput").ap()
    t_wc = nc.dram_tensor("wc", [128, NL, 3, 128], bf16, kind="ExternalInput").ap()
    t_whh = nc.dram_tensor("whh", [128, NL, 3, 128], bf16, kind="ExternalInput").ap()
    t_bhn = nc.dram_tensor("bhn", [128, 8, 128], bf16, kind="ExternalInput").ap()
    t_ind = nc.dram_tensor("ind", [128, 8, B], bf16, kind="ExternalInput").ap()
    t_zpad = nc.dram_tensor("zpad", [128 - KC, 2, NL, B], bf16,
                            kind="ExternalInput").ap()
    t_wout = nc.dram_tensor("wout", [128, F], bf16, kind="ExternalInput").ap()
    t_bout = nc.dram_tensor("bout", [128, F], f32, kind="ExternalInput").ap()
    t_out = nc.dram_tensor("out", [128, 32, F], f32, kind="ExternalOutput").ap()

    with tile.TileContext(nc) as tc:
        with (
            tc.tile_pool(name="const", bufs=1) as cpool,
            tc.tile_pool(name="hpool", bufs=1) as hpool,
            tc.tile_pool(name="stage", bufs=1) as spool,
        ):
            a12 = cpool.tile([128, 4, 2, NL], bf16)
            wc = cpool.tile([128, NL, 3, 128], bf16)
            whh = cpool.tile([128, NL, 3, 128], bf16)
            bhn = cpool.tile([128, 8, 128], bf16)
            ind = cpool.tile([128, 8, B], bf16)
            wout = cpool.tile([128, F], bf16)
            bout = cpool.tile([128, F], f32)
            for sb, dr in [(a12, t_a12), (wc, t_wc), (whh, t_whh),
                           (bhn, t_bhn), (ind, t_ind), (wout, t_wout),
                           (bout, t_bout)]:
                nc.sync.dma_start(sb[:], dr[:])
            h = hpool.tile([128, NL, B], bf16)
            nc.any.memset(h[:], 0.0)

            # staging for diffusion hops, baseline layout [(w%8)*16+f, cc, nl, b]
            xc1 = spool.tile([128, 3, NL, B], bf16)
            xc2 = spool.tile([128, 3, NL, B], bf16)

            # ---- phase 1 (cc-chunked) interleaved with phase 2 ----
            with (
                tc.tile_pool(name="xmb", bufs=2) as xmpool,
                tc.tile_pool(name="p1", bufs=2, space="PSUM") as p1pool,
                tc.tile_pool(name="xs", bufs=2) as xspool,
                tc.tile_pool(name="gp", bufs=2, space="PSUM") as gppool,
                tc.tile_pool(name="pn", bufs=1, space="PSUM") as pnpool,
                tc.tile_pool(name="hn", bufs=1, space="PSUM") as hnpool,
                tc.tile_pool(name="ew", bufs=2) as ewpool,
            ):
                def p1_chunk(cc, bq):
                    # diffusion hops for w-chunk cc, batch pair bq
                    xmb = xmpool.tile([128, 2, 4, 128], bf16, tag="xmb")
                    nc.sync.dma_start(xmb[:], t_xm[cc, :, 2 * bq:2 * bq + 2])
                    P12 = p1pool.tile([128, 2, NL, 2], f32, tag="P12")
                    for bs in range(2):
                        for mc in range(4):
                            nc.tensor.matmul(P12[:, :, :, bs],
                                             xmb[:, bs, mc, :],
                                             a12[:, mc, :, :],
                                             start=(mc == 0), stop=(mc == 3))
                    nc.vector.tensor_copy(xc1[:, cc, :, 2 * bq:2 * bq + 2],
                                          P12[:, 0, :, :])
                    nc.scalar.copy(xc2[:, cc, :, 2 * bq:2 * bq + 2],
                                   P12[:, 1, :, :])

                for bq in range(B // 2):
                    p1_chunk(0, bq)

                for t in range(NT):
                    xs = xspool.tile([128, 2, NL, B], bf16, tag="xs")
                    if t < 2:
                        # rows 49-127 are contracted against zero weight rows;
                        # zero them once per ring buffer so no NaNs flow
                        nc.sync.dma_start(xs[KC:128, :, :, :], t_zpad[:])
                    # hop0 rows 0-15 + ones row 16 straight from DRAM
                    nc.sync.dma_start(xs[0:17, :, :, :], t_x0b[t])
                    # hop1/hop2 rows via SBUF->SBUF partition restructure
                    for wsub in range(2):
                        w = 2 * t + wsub
                        wo, ccw = w % 8, w // 8
                        src1 = xc1[16 * wo:16 * wo + 16, ccw, :, :]
                        src2 = xc2[16 * wo:16 * wo + 16, ccw, :, :]
                        nc.sync.dma_start(xs[17:33, wsub, :, :], src1)
                        nc.sync.dma_start(xs[33:49, wsub, :, :], src2)

                    for wsub in range(2):
                        for g in range(8):
                            ns = slice(8 * g, 8 * g + 8)
                            Prz = gppool.tile([128, 2, 8, B], f32, tag="Prz")
                            Pn = pnpool.tile([128, 8, B], f32, tag="Pn")
                            Phn = hnpool.tile([128, 8, B], f32, tag="Phn")
                            # b_hn via indicator matmul (opens accumulation)
                            nc.tensor.matmul(Phn[:], bhn[:, g, :], ind[:],
                                             start=True, stop=False,
                                             skip_group_check=True)
                            for j in range(8):
                                n = 8 * g + j
                                xsn = xs[:, wsub, n, :]
                                hn_ = h[:, n, :]
                                for gc in range(2):
                                    o = Prz[:, gc, j, :]
                                    nc.tensor.matmul(o, wc[:, n, gc, :], xsn,
                                                     start=True, stop=False)
                                    nc.tensor.matmul(o, whh[:, n, gc, :], hn_,
                                                     start=False, stop=True)
                                nc.tensor.matmul(Pn[:, j, :], wc[:, n, 2, :],
                                                 xsn, start=True, stop=True)
                                nc.tensor.matmul(Phn[:, j, :], whh[:, n, 2, :],
                                                 hn_, start=False,
                                                 stop=(j == 7),
                                                 skip_group_check=True)
                            # ---- gate elementwise (flat APs for 2x DVE) ----
                            hsl = h[:, ns, :].rearrange("p a b -> p (a b)")
                            rz = ewpool.tile([128, 2, 8, B], bf16, tag="rz")
                            nc.scalar.activation(
                                rz[:].rearrange("p a c b -> p (a c b)"),
                                Prz[:].rearrange("p a c b -> p (a c b)"),
                                Act.Sigmoid)
                            r_ = rz[:, 0].rearrange("p a b -> p (a b)")
                            z_ = rz[:, 1].rearrange("p a b -> p (a b)")
                            tt = ewpool.tile([128, 8 * B], bf16, tag="tt")
                            nc.vector.tensor_tensor(
                                tt[:], r_,
                                Phn[:].rearrange("p a b -> p (a b)"), Alu.mult)
                            ut = ewpool.tile([128, 8 * B], bf16, tag="ut")
                            nc.vector.tensor_tensor(
                                ut[:], tt[:],
                                Pn[:].rearrange("p a b -> p (a b)"), Alu.add)
                            nt = ewpool.tile([128, 8 * B], bf16, tag="nt")
                            nc.scalar.activation(nt[:], ut[:], Act.Tanh)
                            st = ewpool.tile([128, 8 * B], bf16, tag="st")
                            nc.gpsimd.tensor_tensor(st[:], hsl, nt[:],
                                                    Alu.subtract)
                            vt = ewpool.tile([128, 8 * B], bf16, tag="vt")
                            nc.vector.tensor_tensor(vt[:], z_, st[:], Alu.mult)
                            nc.vector.tensor_tensor(hsl, nt[:], vt[:], Alu.add)

                    # trickle next w-chunk's diffusion hops under the GRU work
                    if t < 8:
                        ncc = 1 + t // 4
                        for k in range(8):
                            p1_chunk(ncc, 8 * (t % 4) + k)

            # ---- output projection ----
            with (
                tc.tile_pool(name="po", bufs=1, space="PSUM") as popool,
                tc.tile_pool(name="ou", bufs=1) as oupool,
            ):
                Po = popool.tile([128, 32, F], f32)
                for c in range(32):
                    nc.tensor.matmul(Po[:, c, :], h[:, 2 * c:2 * c + 2, :],
                                     wout[:], start=True, stop=True)
                outsb = oupool.tile([128, 32, F], f32)
                nc.vector.tensor_tensor(
                    outsb[:], Po[:], bout[:, None, :].to_broadcast((128, 32, F)),
                    Alu.add)
                nc.sync.dma_start(t_out[:], outsb[:])
    nc.compile()


def kernel(**inputs):
    import concourse.bacc as bacc
    import concourse.bass as bass
    import concourse.mybir as mybir
    import concourse.tile as tile
    from concourse import bass_utils

    x = np.asarray(inputs["x"], np.float32)
    A = np.asarray(inputs["A_fw"], np.float32)
    dcw = np.asarray(inputs["dc_weights"], np.float32)
    W_ih = np.asarray(inputs["W_ih"], np.float32)
    W_hh = np.asarray(inputs["W_hh"], np.float32)
    b_ih = np.asarray(inputs["b_ih"], np.float32)
    b_hh = np.asarray(inputs["b_hh"], np.float32)
    W_out = np.asarray(inputs["W_out"], np.float32)
    b_out = np.asarray(inputs["b_out"], np.float32)

    A2 = A @ A
    dc_all = np.stack([dcw[0:16], dcw[16:32] + dcw[32:48], dcw[48:64] + dcw[64:80]])
    xbf = x.astype(BF)
    xm = np.ascontiguousarray(
        xbf.reshape(B, 4, 128, 3, 128).transpose(3, 2, 0, 1, 4))
    wout_h = W_out.astype(BF)
    bout_h = np.tile(b_out[None, :], (128, 1)).astype(np.float32)

    # fused input weights: Wc[n, gate, j, (hop, f)] = sum_h W_ih[n, gj, h] dc[hop, f, h]
    Wg = W_ih.reshape(N, 3, 128, H)
    wc_full = np.einsum('ngjh,ofh->ngjof', Wg, dc_all)        # [N, 3, 128, 3, 16]
    bias_in = b_ih.reshape(N, 3, 128).copy()
    bias_in[:, 0:2, :] += b_hh.reshape(N, 3, 128)[:, 0:2, :]  # r,z combined bias
    # K-layout: rows 0-15 hop0, 16 ones(bias), 17-32 hop1, 33-48 hop2,
    # 49-127 zero padding (keeps K=128 so fast weight load stays enabled)
    wc_k = np.zeros((N, 3, 128, 128), np.float32)
    wc_k[:, :, 0:16, :] = wc_full[:, :, :, 0, :].transpose(0, 1, 3, 2)
    wc_k[:, :, 16, :] = bias_in
    wc_k[:, :, 17:33, :] = wc_full[:, :, :, 1, :].transpose(0, 1, 3, 2)
    wc_k[:, :, 33:49, :] = wc_full[:, :, :, 2, :].transpose(0, 1, 3, 2)

    ind_h = np.zeros((128, 8, B), np.float32)
    for k in range(8):
        ind_h[k, k, :] = 1.0
    ind_h = ind_h.astype(BF)

    in_maps = []
    for c in range(8):
        ns = slice(c * NL, (c + 1) * NL)
        a1t = A[ns].T.astype(BF).reshape(4, 128, NL).transpose(1, 0, 2)
        a12 = np.ascontiguousarray(np.stack([a1t, A2[ns].T.astype(BF)
                                             .reshape(4, 128, NL)
                                             .transpose(1, 0, 2)], axis=2))
        xl = xbf[:, ns]  # [b, nl, w, f]
        # x0b[t, p, wsub, nl, b]: p 0-15 = f rows of x[w=2t+wsub], p16 = ones
        x0b = np.empty((NT, 17, 2, NL, B), np.float32)
        xw = np.asarray(xl, np.float32).transpose(2, 3, 1, 0)  # [w, f, nl, b]
        x0b[:, 0:16] = xw.reshape(NT, 2, 16, NL, B).transpose(0, 2, 1, 3, 4)
        x0b[:, 16] = 1.0
        wc_h = np.ascontiguousarray(
            wc_k[ns].transpose(2, 0, 1, 3)).astype(BF)       # [128, NL, 3, 128]
        whh_h = np.ascontiguousarray(
            W_hh[ns].transpose(2, 0, 1).astype(BF).reshape(128, NL, 3, 128))
        bhn_h = np.zeros((128, 8, 128), np.float32)           # [n8(+pad), g, j]
        bhn_h[0:8] = b_hh[ns, 256:384].reshape(8, 8, 128).transpose(1, 0, 2)
        bhn_h = bhn_h.astype(BF)
        in_maps.append({
            "a12": a12, "xm": xm,
            "x0b": np.ascontiguousarray(x0b).astype(BF),
            "wc": wc_h, "whh": whh_h, "bhn": bhn_h, "ind": ind_h,
            "zpad": np.zeros((128 - KC, 2, NL, B), BF),
            "wout": wout_h, "bout": bout_h,
        })

    nc = bacc.Bacc("TRN2", target_bir_lowering=False, debug=False, num_devices=8)
    _build(nc, bass, mybir, tile)
    import os, time
    trace = bool(os.environ.get("DGCN_TRACE"))
    res = bass_utils.run_bass_kernel_spmd(nc, in_maps, core_ids=list(range(8)),
                                          trace=trace)
    if trace and res.exec_time_ns:
        print(f"MEASURED exec_time_ns: {res.exec_time_ns}", flush=True)
        try:
            with open("/tmp/dgcn_exec_ns.txt", "w") as f:
                f.write(str(res.exec_time_ns))
        except Exception:
            pass
        if res.instructions_and_trace:
            print(f"trace: {res.instructions_and_trace[1]}", flush=True)
    if os.environ.get("DGCN_BENCH"):
        for it in range(int(os.environ["DGCN_BENCH"])):
            t0 = time.time()
            res = bass_utils.run_bass_kernel_spmd(nc, in_maps, core_ids=list(range(8)))
            print(f"bench iter {it}: {(time.time()-t0)*1e3:.1f} ms", flush=True)

    out = np.zeros((B, N, F), np.float32)
    for c in range(8):
        arr = res.results[c]["out"]  # [128, 32, F]
        tmp = arr.transpose(1, 0, 2).reshape(32, 2, B, F).transpose(2, 0, 1, 3)
        out[:, c * NL:(c + 1) * NL] = tmp.reshape(B, NL, F)
    return out


# revision 34
# speedup vs baseline: 1.1458x; 1.0061x over previous
import numpy as np
import ml_dtypes

BF = ml_dtypes.bfloat16
B, N, WT, F, H = 64, 512, 24, 16, 128
NL = N // 8   # nodes per core
NT = WT // 2  # timestep pairs
KC = 49       # fused input contraction: 3 hops * 16 f + ones row

# xstack partition layout (rows of the K=49 contraction):
#   p 0-15  : hop0 (raw x features)
#   p 16    : ones (carries all input-side biases)
#   p 17-32 : hop1 (A @ x features)
#   p 33-48 : hop2 (A^2 @ x features)


def _build(nc, bass, mybir, tile):
    f32 = mybir.dt.float32
    bf16 = mybir.dt.bfloat16
    Alu = mybir.AluOpType
    Act = mybir.ActivationFunctionType

    t_a12 = nc.dram_tensor("a12", [128, 4, 2, NL], bf16, kind="ExternalInput").ap()
    t_xm = nc.dram_tensor("xm", [3, 128, B, 4, 128], bf16, kind="ExternalInput").ap()
    # hop0 features + ones row, per timestep pair: [pair, 17, 2, nl, b]
    t_x0b = nc.dram_tensor("x0b", [NT, 17, 2, NL, B], bf16, kind="ExternalInput").ap()
    t_wc = nc.dram_tensor("wc", [128, NL, 3, 128], bf16, kind="ExternalInput").ap()
    t_whh = nc.dram_tensor("whh", [128, NL, 3, 128], bf16, kind="ExternalInput").ap()
    t_bhn = nc.dram_tensor("bhn", [128, 8, 128], bf16, kind="ExternalInput").ap()
    t_ind = nc.dram_tensor("ind", [128, 8, B], bf16, kind="ExternalInput").ap()
    t_zpad = nc.dram_tensor("zpad", [128 - KC, 2, NL, B], bf16,
                            kind="ExternalInput").ap()
    t_wout = nc.dram_tensor("wout", [128, F], bf16, kind="ExternalInput").ap()
    t_bout = nc.dram_tensor("bout", [128, F], f32, kind="ExternalInput").ap()
    t_out = nc.dram_tensor("out", [128, 32, F], f32, kind="ExternalOutput").ap()

    with tile.TileContext(nc) as tc:
        with (
            tc.tile_pool(name="const", bufs=1) as cpool,
            tc.tile_pool(name="hpool", bufs=1) as hpool,
            tc.tile_pool(name="stage", bufs=1) as spool,
        ):
            a12 = cpool.tile([128, 4, 2, NL], bf16)
            wc = cpool.tile([128, NL, 3, 128], bf16)
            whh = cpool.tile([128, NL, 3, 128], bf16)
            bhn = cpool.tile([128, 8, 128], bf16)
            ind = cpool.tile([128, 8, B], bf16)
            wout = cpool.tile([128, F], bf16)
            bout = cpool.tile([128, F], f32)
            for sb, dr in [(a12, t_a12), (bhn, t_bhn), (ind, t_ind),
                           (wout, t_wout), (bout, t_bout)]:
                nc.sync.dma_start(sb[:], dr[:])
            h = hpool.tile([128, NL, B], bf16)
            nc.any.memset(h[:], 0.0)

            # staging for diffusion hops, baseline layout [(w%8)*16+f, cc, nl, b]
            xc1 = spool.tile([128, 3, NL, B], bf16)
            xc2 = spool.tile([128, 3, NL, B], bf16)

            # ---- phase 1: diffusion hops, chunked and DMA-pipelined ----
            with (
                tc.tile_pool(name="xmb", bufs=4) as xmpool,
                tc.tile_pool(name="p1", bufs=2, space="PSUM") as p1pool,
            ):
                for cc in range(3):
                    for bq in range(B // 2):
                        xmb = xmpool.tile([128, 2, 4, 128], bf16, tag="xmb")
                        nc.sync.dma_start(xmb[:],
                                          t_xm[cc, :, 2 * bq:2 * bq + 2])
                        P12 = p1pool.tile([128, 2, NL, 2], f32, tag="P12")
                        for bs in range(2):
                            for mc in range(4):
                                nc.tensor.matmul(P12[:, :, :, bs],
                                                 xmb[:, bs, mc, :],
                                                 a12[:, mc, :, :],
                                                 start=(mc == 0),
                                                 stop=(mc == 3))
                        nc.vector.tensor_copy(
                            xc1[:, cc, :, 2 * bq:2 * bq + 2], P12[:, 0, :, :])
                        nc.scalar.copy(
                            xc2[:, cc, :, 2 * bq:2 * bq + 2], P12[:, 1, :, :])

            # heavy GRU weights arrive while phase 1 computes
            for sb, dr in [(wc, t_wc), (whh, t_whh)]:
                nc.sync.dma_start(sb[:], dr[:])

            # ---- phase 2: GRU over time, input projection fused into Wc ----
            with (
                tc.tile_pool(name="xs", bufs=2) as xspool,
                tc.tile_pool(name="gp", bufs=2, space="PSUM") as gppool,
                tc.tile_pool(name="pn", bufs=2, space="PSUM") as pnpool,
                tc.tile_pool(name="hn", bufs=2, space="PSUM") as hnpool,
                tc.tile_pool(name="ew", bufs=2) as ewpool,
            ):
                for t in range(NT):
                    xs = xspool.tile([128, 2, NL, B], bf16, tag="xs")
                    if t < 2:
                        # rows 49-127 are contracted against zero weight rows;
                        # zero them once per ring buffer so no NaNs flow
                        nc.sync.dma_start(xs[KC:128, :, :, :], t_zpad[:])
                    # hop0 rows 0-15 + ones row 16 straight from DRAM
                    nc.sync.dma_start(xs[0:17, :, :, :], t_x0b[t])
                    # hop1/hop2 rows via SBUF->SBUF partition restructure
                    for wsub in range(2):
                        w = 2 * t + wsub
                        wo, ccw = w % 8, w // 8
                        src1 = xc1[16 * wo:16 * wo + 16, ccw, :, :]
                        src2 = xc2[16 * wo:16 * wo + 16, ccw, :, :]
                        nc.sync.dma_start(xs[17:33, wsub, :, :], src1)
                        nc.sync.dma_start(xs[33:49, wsub, :, :], src2)

                    for wsub in range(2):
                        for g in range(8):
                            ns = slice(8 * g, 8 * g + 8)
                            Prz = gppool.tile([128, 2, 8, B], f32, tag="Prz")
                            Pn = pnpool.tile([128, 8, B], f32, tag="Pn")
                            Phn = hnpool.tile([128, 8, B], f32, tag="Phn")
                            # b_hn via indicator matmul (opens accumulation)
                            nc.tensor.matmul(Phn[:], bhn[:, g, :], ind[:],
                                             start=True, stop=False,
                                             skip_group_check=True)
                            for j in range(8):
                                n = 8 * g + j
                                xsn = xs[:, wsub, n, :]
                                hn_ = h[:, n, :]
                                for gc in range(2):
                                    o = Prz[:, gc, j, :]
                                    nc.tensor.matmul(o, wc[:, n, gc, :], xsn,
                                                     start=True, stop=False)
                                    nc.tensor.matmul(o, whh[:, n, gc, :], hn_,
                                                     start=False, stop=True)
                                nc.tensor.matmul(Pn[:, j, :], wc[:, n, 2, :],
                                                 xsn, start=True, stop=True)
                                nc.tensor.matmul(Phn[:, j, :], whh[:, n, 2, :],
                                                 hn_, start=False,
                                                 stop=(j == 7),
                                                 skip_group_check=True)
                            # ---- gate elementwise (flat APs for 2x DVE) ----
                            hsl = h[:, ns, :].rearrange("p a b -> p (a b)")
                            rz = ewpool.tile([128, 2, 8, B], bf16, tag="rz")
                            nc.scalar.activation(
                                rz[:].rearrange("p a c b -> p (a c b)"),
                                Prz[:].rearrange("p a c b -> p (a c b)"),
                                Act.Sigmoid)
                            r_ = rz[:, 0].rearrange("p a b -> p (a b)")
                            z_ = rz[:, 1].rearrange("p a b -> p (a b)")
                            tt = ewpool.tile([128, 8 * B], bf16, tag="tt")
                            nc.vector.tensor_tensor(
                                tt[:], r_,
                                Phn[:].rearrange("p a b -> p (a b)"), Alu.mult)
                            ut = ewpool.tile([128, 8 * B], bf16, tag="ut")
                            nc.vector.tensor_tensor(
                                ut[:], tt[:],
                                Pn[:].rearrange("p a b -> p (a b)"), Alu.add)
                            nt = ewpool.tile([128, 8 * B], bf16, tag="nt")
                            nc.scalar.activation(nt[:], ut[:], Act.Tanh)
                            st = ewpool.tile([128, 8 * B], bf16, tag="st")
                            nc.gpsimd.tensor_tensor(st[:], hsl, nt[:],
                                                    Alu.subtract)
                            vt = ewpool.tile([128, 8 * B], bf16, tag="vt")
                            nc.vector.tensor_tensor(vt[:], z_, st[:], Alu.mult)
                            nc.vector.tensor_tensor(hsl, nt[:], vt[:], Alu.add)

            # ---- output projection ----
            with (
                tc.tile_pool(name="po", bufs=1, space="PSUM") as popool,
                tc.tile_pool(name="ou", bufs=1) as oupool,
            ):
                Po = popool.tile([128, 32, F], f32)
                for c in range(32):
                    nc.tensor.matmul(Po[:, c, :], h[:, 2 * c:2 * c + 2, :],
                                     wout[:], start=True, stop=True)
                outsb = oupool.tile([128, 32, F], f32)
                nc.vector.tensor_tensor(
                    outsb[:], Po[:], bout[:, None, :].to_broadcast((128, 32, F)),
                    Alu.add)
                nc.sync.dma_start(t_out[:], outsb[:])
    nc.compile()


def kernel(**inputs):
    import concourse.bacc as bacc
    import concourse.bass as bass
    import concourse.mybir as mybir
    import concourse.tile as tile
    from concourse import bass_utils

    x = np.asarray(inputs["x"], np.float32)
    A = np.asarray(inputs["A_fw"], np.float32)
    dcw = np.asarray(inputs["dc_weights"], np.float32)
    W_ih = np.asarray(inputs["W_ih"], np.float32)
    W_hh = np.asarray(inputs["W_hh"], np.float32)
    b_ih = np.asarray(inputs["b_ih"], np.float32)
    b_hh = np.asarray(inputs["b_hh"], np.float32)
    W_out = np.asarray(inputs["W_out"], np.float32)
    b_out = np.asarray(inputs["b_out"], np.float32)

    A2 = A @ A
    dc_all = np.stack([dcw[0:16], dcw[16:32] + dcw[32:48], dcw[48:64] + dcw[64:80]])
    xbf = x.astype(BF)
    xm = np.ascontiguousarray(
        xbf.reshape(B, 4, 128, 3, 128).transpose(3, 2, 0, 1, 4))
    wout_h = W_out.astype(BF)
    bout_h = np.tile(b_out[None, :], (128, 1)).astype(np.float32)

    # fused input weights: Wc[n, gate, j, (hop, f)] = sum_h W_ih[n, gj, h] dc[hop, f, h]
    Wg = W_ih.reshape(N, 3, 128, H)
    wc_full = np.einsum('ngjh,ofh->ngjof', Wg, dc_all)        # [N, 3, 128, 3, 16]
    bias_in = b_ih.reshape(N, 3, 128).copy()
    bias_in[:, 0:2, :] += b_hh.reshape(N, 3, 128)[:, 0:2, :]  # r,z combined bias
    # K-layout: rows 0-15 hop0, 16 ones(bias), 17-32 hop1, 33-48 hop2,
    # 49-127 zero padding (keeps K=128 so fast weight load stays enabled)
    wc_k = np.zeros((N, 3, 128, 128), np.float32)
    wc_k[:, :, 0:16, :] = wc_full[:, :, :, 0, :].transpose(0, 1, 3, 2)
    wc_k[:, :, 16, :] = bias_in
    wc_k[:, :, 17:33, :] = wc_full[:, :, :, 1, :].transpose(0, 1, 3, 2)
    wc_k[:, :, 33:49, :] = wc_full[:, :, :, 2, :].transpose(0, 1, 3, 2)

    ind_h = np.zeros((128, 8, B), np.float32)
    for k in range(8):
        ind_h[k, k, :] = 1.0
    ind_h = ind_h.astype(BF)

    in_maps = []
    for c in range(8):
        ns = slice(c * NL, (c + 1) * NL)
        a1t = A[ns].T.astype(BF).reshape(4, 128, NL).transpose(1, 0, 2)
        a12 = np.ascontiguousarray(np.stack([a1t, A2[ns].T.astype(BF)
                                             .reshape(4, 128, NL)
                                             .transpose(1, 0, 2)], axis=2))
        xl = xbf[:, ns]  # [b, nl, w, f]
        # x0b[t, p, wsub, nl, b]: p 0-15 = f rows of x[w=2t+wsub], p16 = ones
        x0b = np.empty((NT, 17, 2, NL, B), np.float32)
        xw = np.asarray(xl, np.float32).transpose(2, 3, 1, 0)  # [w, f, nl, b]
        x0b[:, 0:16] = xw.reshape(NT, 2, 16, NL, B).transpose(0, 2, 1, 3, 4)
        x0b[:, 16] = 1.0
        wc_h = np.ascontiguousarray(
            wc_k[ns].transpose(2, 0, 1, 3)).astype(BF)       # [128, NL, 3, 128]
        whh_h = np.ascontiguousarray(
            W_hh[ns].transpose(2, 0, 1).astype(BF).reshape(128, NL, 3, 128))
        bhn_h = np.zeros((128, 8, 128), np.float32)           # [n8(+pad), g, j]
        bhn_h[0:8] = b_hh[ns, 256:384].reshape(8, 8, 128).transpose(1, 0, 2)
        bhn_h = bhn_h.astype(BF)
        in_maps.append({
            "a12": a12, "xm": xm,
            "x0b": np.ascontiguousarray(x0b).astype(BF),
            "wc": wc_h, "whh": whh_h, "bhn": bhn_h, "ind": ind_h,
            "zpad": np.zeros((128 - KC, 2, NL, B), BF),
            "wout": wout_h, "bout": bout_h,
        })

    nc = bacc.Bacc("TRN2", target_bir_lowering=False, debug=False, num_devices=8)
    _build(nc, bass, mybir, tile)
    import os, time
    trace = bool(os.environ.get("DGCN_TRACE"))
    res = bass_utils.run_bass_kernel_spmd(nc, in_maps, core_ids=list(range(8)),
                                          trace=trace)
    if trace and res.exec_time_ns:
        print(f"MEASURED exec_time_ns: {res.exec_time_ns}", flush=True)
        try:
            with open("/tmp/dgcn_exec_ns.txt", "w") as f:
                f.write(str(res.exec_time_ns))
        except Exception:
            pass
        if res.instructions_and_trace:
            print(f"trace: {res.instructions_and_trace[1]}", flush=True)
    if os.environ.get("DGCN_BENCH"):
        for it in range(int(os.environ["DGCN_BENCH"])):
            t0 = time.time()
            res = bass_utils.run_bass_kernel_spmd(nc, in_maps, core_ids=list(range(8)))
            print(f"bench iter {it}: {(time.time()-t0)*1e3:.1f} ms", flush=True)

    out = np.zeros((B, N, F), np.float32)
    for c in range(8):
        arr = res.results[c]["out"]  # [128, 32, F]
        tmp = arr.transpose(1, 0, 2).reshape(32, 2, B, F).transpose(2, 0, 1, 3)
        out[:, c * NL:(c + 1) * NL] = tmp.reshape(B, NL, F)
    return out


# revision 35
# speedup vs baseline: 1.2234x; 1.0678x over previous
import numpy as np
import ml_dtypes

BF = ml_dtypes.bfloat16
B, N, WT, F, H = 64, 512, 24, 16, 128
NL = N // 8   # nodes per core
NT = WT // 2  # timestep pairs
KC = 49       # fused input contraction: 3 hops * 16 f + ones row

# xstack partition layout (rows of the K=49 contraction):
#   p 0-15  : hop0 (raw x features)
#   p 16    : ones (carries all input-side biases)
#   p 17-32 : hop1 (A @ x features)
#   p 33-48 : hop2 (A^2 @ x features)


def _build(nc, bass, mybir, tile):
    f32 = mybir.dt.float32
    bf16 = mybir.dt.bfloat16
    Alu = mybir.AluOpType
    Act = mybir.ActivationFunctionType

    t_a12 = nc.dram_tensor("a12", [128, 4, 2, NL], bf16, kind="ExternalInput").ap()
    t_xm = nc.dram_tensor("xm", [3, 128, B, 4, 128], bf16, kind="ExternalInput").ap()
    # hop0 features + ones row, per timestep pair: [pair, 17, 2, nl, b]
    t_x0b = nc.dram_tensor("x0b", [NT, 17, 2, NL, B], bf16, kind="ExternalInput").ap()
    t_wc = nc.dram_tensor("wc", [128, NL, 3, 128], bf16, kind="ExternalInput").ap()
    t_whh = nc.dram_tensor("whh", [128, NL, 3, 128], bf16, kind="ExternalInput").ap()
    t_bhn = nc.dram_tensor("bhn", [128, 8, 128], bf16, kind="ExternalInput").ap()
    t_ind = nc.dram_tensor("ind", [128, 8, B], bf16, kind="ExternalInput").ap()
    t_zpad = nc.dram_tensor("zpad", [128 - KC, 2, NL, B], bf16,
                            kind="ExternalInput").ap()
    t_wout = nc.dram_tensor("wout", [128, F], bf16, kind="ExternalInput").ap()
    t_bout = nc.dram_tensor("bout", [128, F], f32, kind="ExternalInput").ap()
    t_out = nc.dram_tensor("out", [128, 32, F], f32, kind="ExternalOutput").ap()

    with tile.TileContext(nc) as tc:
        with (
            tc.tile_pool(name="const", bufs=1) as cpool,
            tc.tile_pool(name="hpool", bufs=1) as hpool,
            tc.tile_pool(name="stage", bufs=1) as spool,
        ):
            a12 = cpool.tile([128, 4, 2, NL], bf16)
            wc = cpool.tile([128, NL, 3, 128], bf16)
            whh = cpool.tile([128, NL, 3, 128], bf16)
            bhn = cpool.tile([128, 8, 128], bf16)
            ind = cpool.tile([128, 8, B], bf16)
            wout = cpool.tile([128, F], bf16)
            bout = cpool.tile([128, F], f32)
            for sb, dr in [(a12, t_a12), (bhn, t_bhn), (ind, t_ind),
                           (wout, t_wout), (bout, t_bout)]:
                nc.sync.dma_start(sb[:], dr[:])
            h = hpool.tile([128, NL, B], bf16)
            nc.any.memset(h[:], 0.0)

            # staging for diffusion hops, baseline layout [(w%8)*16+f, cc, nl, b]
            xc1 = spool.tile([128, 3, NL, B], bf16)
            xc2 = spool.tile([128, 3, NL, B], bf16)

            # ---- phase 1: diffusion hops, chunked and DMA-pipelined ----
            with (
                tc.tile_pool(name="xmb", bufs=4) as xmpool,
                tc.tile_pool(name="p1", bufs=2, space="PSUM") as p1pool,
            ):
                for cc in range(3):
                    for bq in range(B // 4):
                        xmb = xmpool.tile([128, 4, 4, 128], bf16, tag="xmb")
                        nc.sync.dma_start(xmb[:],
                                          t_xm[cc, :, 4 * bq:4 * bq + 4])
                        P12 = p1pool.tile([128, 2, NL, 4], f32, tag="P12")
                        for bs in range(4):
                            for mc in range(4):
                                nc.tensor.matmul(P12[:, :, :, bs],
                                                 xmb[:, bs, mc, :],
                                                 a12[:, mc, :, :],
                                                 start=(mc == 0),
                                                 stop=(mc == 3))
                        nc.vector.tensor_copy(
                            xc1[:, cc, :, 4 * bq:4 * bq + 4], P12[:, 0, :, :])
                        nc.scalar.copy(
                            xc2[:, cc, :, 4 * bq:4 * bq + 4], P12[:, 1, :, :])

            # heavy GRU weights arrive while phase 1 computes; slice the
            # transfers so they spread across DMA engines
            for sb, dr in [(wc, t_wc), (whh, t_whh)]:
                for sl in range(8):
                    nc.sync.dma_start(sb[:, 8 * sl:8 * sl + 8, :, :],
                                      dr[:, 8 * sl:8 * sl + 8, :, :])

            # ---- phase 2: GRU over time, input projection fused into Wc ----
            with (
                tc.tile_pool(name="xs", bufs=2) as xspool,
                tc.tile_pool(name="gp", bufs=2, space="PSUM") as gppool,
                tc.tile_pool(name="pn", bufs=2, space="PSUM") as pnpool,
                tc.tile_pool(name="hn", bufs=2, space="PSUM") as hnpool,
                tc.tile_pool(name="ew", bufs=2) as ewpool,
            ):
                for t in range(NT):
                    xs = xspool.tile([128, 2, NL, B], bf16, tag="xs")
                    if t < 2:
                        # rows 49-127 are contracted against zero weight rows;
                        # zero them once per ring buffer so no NaNs flow
                        nc.sync.dma_start(xs[KC:128, :, :, :], t_zpad[:])
                    # hop0 rows 0-15 + ones row 16 straight from DRAM
                    nc.sync.dma_start(xs[0:17, :, :, :], t_x0b[t])
                    # hop1/hop2 rows via SBUF->SBUF partition restructure
                    for wsub in range(2):
                        w = 2 * t + wsub
                        wo, ccw = w % 8, w // 8
                        src1 = xc1[16 * wo:16 * wo + 16, ccw, :, :]
                        src2 = xc2[16 * wo:16 * wo + 16, ccw, :, :]
                        nc.sync.dma_start(xs[17:33, wsub, :, :], src1)
                        nc.sync.dma_start(xs[33:49, wsub, :, :], src2)

                    for wsub in range(2):
                        for g in range(8):
                            ns = slice(8 * g, 8 * g + 8)
                            Prz = gppool.tile([128, 2, 8, B], f32, tag="Prz")
                            Pn = pnpool.tile([128, 8, B], f32, tag="Pn")
                            Phn = hnpool.tile([128, 8, B], f32, tag="Phn")
                            # b_hn via indicator matmul (opens accumulation)
                            nc.tensor.matmul(Phn[:], bhn[:, g, :], ind[:],
                                             start=True, stop=False,
                                             skip_group_check=True)
                            for j in range(8):
                                n = 8 * g + j
                                xsn = xs[:, wsub, n, :]
                                hn_ = h[:, n, :]
                                for gc in range(2):
                                    o = Prz[:, gc, j, :]
                                    nc.tensor.matmul(o, wc[:, n, gc, :], xsn,
                                                     start=True, stop=False)
                                    nc.tensor.matmul(o, whh[:, n, gc, :], hn_,
                                                     start=False, stop=True)
                                nc.tensor.matmul(Pn[:, j, :], wc[:, n, 2, :],
                                                 xsn, start=True, stop=True)
                                nc.tensor.matmul(Phn[:, j, :], whh[:, n, 2, :],
                                                 hn_, start=False,
                                                 stop=(j == 7),
                                                 skip_group_check=True)
                            # ---- gate elementwise (flat APs for 2x DVE) ----
                            hsl = h[:, ns, :].rearrange("p a b -> p (a b)")
                            rz = ewpool.tile([128, 2, 8, B], bf16, tag="rz")
                            r_ = rz[:, 0].rearrange("p a b -> p (a b)")
                            z_ = rz[:, 1].rearrange("p a b -> p (a b)")
                            nc.scalar.activation(
                                r_, Prz[:, 0].rearrange("p a b -> p (a b)"),
                                Act.Sigmoid)
                            nc.scalar.activation(
                                z_, Prz[:, 1].rearrange("p a b -> p (a b)"),
                                Act.Sigmoid)
                            tt = ewpool.tile([128, 8 * B], bf16, tag="tt")
                            nc.vector.tensor_tensor(
                                tt[:], r_,
                                Phn[:].rearrange("p a b -> p (a b)"), Alu.mult)
                            ut = ewpool.tile([128, 8 * B], bf16, tag="ut")
                            nc.vector.tensor_tensor(
                                ut[:], tt[:],
                                Pn[:].rearrange("p a b -> p (a b)"), Alu.add)
                            nt = ewpool.tile([128, 8 * B], bf16, tag="nt")
                            nc.scalar.activation(nt[:], ut[:], Act.Tanh)
                            st = ewpool.tile([128, 8 * B], bf16, tag="st")
                            nc.gpsimd.tensor_tensor(st[:], hsl, nt[:],
                                                    Alu.subtract)
                            vt = ewpool.tile([128, 8 * B], bf16, tag="vt")
                            nc.vector.tensor_tensor(vt[:], z_, st[:], Alu.mult)
                            nc.vector.tensor_tensor(hsl, nt[:], vt[:], Alu.add)

            # ---- output projection ----
            with (
                tc.tile_pool(name="po", bufs=1, space="PSUM") as popool,
                tc.tile_pool(name="ou", bufs=1) as oupool,
            ):
                Po = popool.tile([128, 32, F], f32)
                for c in range(32):
                    nc.tensor.matmul(Po[:, c, :], h[:, 2 * c:2 * c + 2, :],
                                     wout[:], start=True, stop=True)
                outsb = oupool.tile([128, 32, F], f32)
                nc.vector.tensor_tensor(
                    outsb[:], Po[:], bout[:, None, :].to_broadcast((128, 32, F)),
                    Alu.add)
                nc.sync.dma_start(t_out[:], outsb[:])
    nc.compile()


def kernel(**inputs):
    import concourse.bacc as bacc
    import concourse.bass as bass
    import concourse.mybir as mybir
    import concourse.tile as tile
    from concourse import bass_utils

    x = np.asarray(inputs["x"], np.float32)
    A = np.asarray(inputs["A_fw"], np.float32)
    dcw = np.asarray(inputs["dc_weights"], np.float32)
    W_ih = np.asarray(inputs["W_ih"], np.float32)
    W_hh = np.asarray(inputs["W_hh"], np.float32)
    b_ih = np.asarray(inputs["b_ih"], np.float32)
    b_hh = np.asarray(inputs["b_hh"], np.float32)
    W_out = np.asarray(inputs["W_out"], np.float32)
    b_out = np.asarray(inputs["b_out"], np.float32)

    A2 = A @ A
    dc_all = np.stack([dcw[0:16], dcw[16:32] + dcw[32:48], dcw[48:64] + dcw[64:80]])
    xbf = x.astype(BF)
    xm = np.ascontiguousarray(
        xbf.reshape(B, 4, 128, 3, 128).transpose(3, 2, 0, 1, 4))
    wout_h = W_out.astype(BF)
    bout_h = np.tile(b_out[None, :], (128, 1)).astype(np.float32)

    # fused input weights: Wc[n, gate, j, (hop, f)] = sum_h W_ih[n, gj, h] dc[hop, f, h]
    Wg = W_ih.reshape(N, 3, 128, H)
    wc_full = np.einsum('ngjh,ofh->ngjof', Wg, dc_all)        # [N, 3, 128, 3, 16]
    bias_in = b_ih.reshape(N, 3, 128).copy()
    bias_in[:, 0:2, :] += b_hh.reshape(N, 3, 128)[:, 0:2, :]  # r,z combined bias
    # K-layout: rows 0-15 hop0, 16 ones(bias), 17-32 hop1, 33-48 hop2,
    # 49-127 zero padding (keeps K=128 so fast weight load stays enabled)
    wc_k = np.zeros((N, 3, 128, 128), np.float32)
    wc_k[:, :, 0:16, :] = wc_full[:, :, :, 0, :].transpose(0, 1, 3, 2)
    wc_k[:, :, 16, :] = bias_in
    wc_k[:, :, 17:33, :] = wc_full[:, :, :, 1, :].transpose(0, 1, 3, 2)
    wc_k[:, :, 33:49, :] = wc_full[:, :, :, 2, :].transpose(0, 1, 3, 2)

    ind_h = np.zeros((128, 8, B), np.float32)
    for k in range(8):
        ind_h[k, k, :] = 1.0
    ind_h = ind_h.astype(BF)

    in_maps = []
    for c in range(8):
        ns = slice(c * NL, (c + 1) * NL)
        a1t = A[ns].T.astype(BF).reshape(4, 128, NL).transpose(1, 0, 2)
        a12 = np.ascontiguousarray(np.stack([a1t, A2[ns].T.astype(BF)
                                             .reshape(4, 128, NL)
                                             .transpose(1, 0, 2)], axis=2))
        xl = xbf[:, ns]  # [b, nl, w, f]
        # x0b[t, p, wsub, nl, b]: p 0-15 = f rows of x[w=2t+wsub], p16 = ones
        x0b = np.empty((NT, 17, 2, NL, B), np.float32)
        xw = np.asarray(xl, np.float32).transpose(2, 3, 1, 0)  # [w, f, nl, b]
        x0b[:, 0:16] = xw.reshape(NT, 2, 16, NL, B).transpose(0, 2, 1, 3, 4)
        x0b[:, 16] = 1.0
        wc_h = np.ascontiguousarray(
            wc_k[ns].transpose(2, 0, 1, 3)).astype(BF)       # [128, NL, 3, 128]
        whh_h = np.ascontiguousarray(
            W_hh[ns].transpose(2, 0, 1).astype(BF).reshape(128, NL, 3, 128))
        bhn_h = np.zeros((128, 8, 128), np.float32)           # [n8(+pad), g, j]
        bhn_h[0:8] = b_hh[ns, 256:384].reshape(8, 8, 128).transpose(1, 0, 2)
        bhn_h = bhn_h.astype(BF)
        in_maps.append({
            "a12": a12, "xm": xm,
            "x0b": np.ascontiguousarray(x0b).astype(BF),
            "wc": wc_h, "whh": whh_h, "bhn": bhn_h, "ind": ind_h,
            "zpad": np.zeros((128 - KC, 2, NL, B), BF),
            "wout": wout_h, "bout": bout_h,
        })

    nc = bacc.Bacc("TRN2", target_bir_lowering=False, debug=False, num_devices=8)
    _build(nc, bass, mybir, tile)
    import os, time
    trace = bool(os.environ.get("DGCN_TRACE"))
    res = bass_utils.run_bass_kernel_spmd(nc, in_maps, core_ids=list(range(8)),
                                          trace=trace)
    if trace and res.exec_time_ns:
        print(f"MEASURED exec_time_ns: {res.exec_time_ns}", flush=True)
        try:
            with open("/tmp/dgcn_exec_ns.txt", "w") as f:
                f.write(str(res.exec_time_ns))
        except Exception:
            pass
        if res.instructions_and_trace:
            print(f"trace: {res.instructions_and_trace[1]}", flush=True)
    if os.environ.get("DGCN_BENCH"):
        for it in range(int(os.environ["DGCN_BENCH"])):
            t0 = time.time()
            res = bass_utils.run_bass_kernel_spmd(nc, in_maps, core_ids=list(range(8)))
            print(f"bench iter {it}: {(time.time()-t0)*1e3:.1f} ms", flush=True)

    out = np.zeros((B, N, F), np.float32)
    for c in range(8):
        arr = res.results[c]["out"]  # [128, 32, F]
        tmp = arr.transpose(1, 0, 2).reshape(32, 2, B, F).transpose(2, 0, 1, 3)
        out[:, c * NL:(c + 1) * NL] = tmp.reshape(B, NL, F)
    return out


# revision 36
# speedup vs baseline: 1.3780x; 1.1263x over previous
import numpy as np
import ml_dtypes

BF = ml_dtypes.bfloat16
B, N, WT, F, H = 64, 512, 24, 16, 128
NL = N // 8   # nodes per core
NT = WT // 2  # timestep pairs
KC = 49       # fused input contraction: 3 hops * 16 f + ones row

# xstack partition layout (rows of the K=49 contraction):
#   p 0-15  : hop0 (raw x features)
#   p 16    : ones (carries all input-side biases)
#   p 17-32 : hop1 (A @ x features)
#   p 33-48 : hop2 (A^2 @ x features)


def _build(nc, bass, mybir, tile):
    f32 = mybir.dt.float32
    bf16 = mybir.dt.bfloat16
    Alu = mybir.AluOpType
    Act = mybir.ActivationFunctionType

    t_a12 = nc.dram_tensor("a12", [128, 4, 2, NL], bf16, kind="ExternalInput").ap()
    t_xm = nc.dram_tensor("xm", [3, 128, B, 4, 128], bf16, kind="ExternalInput").ap()
    # hop0 features + ones row, per timestep pair: [pair, 17, 2, nl, b]
    t_x0b = nc.dram_tensor("x0b", [NT, 17, 2, NL, B], bf16, kind="ExternalInput").ap()
    t_wc = nc.dram_tensor("wc", [128, NL, 3, 128], bf16, kind="ExternalInput").ap()
    t_whh = nc.dram_tensor("whh", [128, NL, 3, 128], bf16, kind="ExternalInput").ap()
    t_bhn = nc.dram_tensor("bhn", [128, 8, 128], bf16, kind="ExternalInput").ap()
    t_ind = nc.dram_tensor("ind", [128, 8, B], bf16, kind="ExternalInput").ap()
    t_zpad = nc.dram_tensor("zpad", [128 - KC, 2, NL, B], bf16,
                            kind="ExternalInput").ap()
    t_wout = nc.dram_tensor("wout", [128, F], bf16, kind="ExternalInput").ap()
    t_bout = nc.dram_tensor("bout", [128, F], f32, kind="ExternalInput").ap()
    t_out = nc.dram_tensor("out", [128, 32, F], f32, kind="ExternalOutput").ap()

    with tile.TileContext(nc) as tc:
        with (
            tc.tile_pool(name="const", bufs=1) as cpool,
            tc.tile_pool(name="hpool", bufs=1) as hpool,
            tc.tile_pool(name="stage", bufs=1) as spool,
        ):
            a12 = cpool.tile([128, 4, 2, NL], bf16)
            wc = cpool.tile([128, NL, 3, 128], bf16)
            whh = cpool.tile([128, NL, 3, 128], bf16)
            bhn = cpool.tile([128, 8, 128], bf16)
            ind = cpool.tile([128, 8, B], bf16)
            wout = cpool.tile([128, F], bf16)
            bout = cpool.tile([128, F], f32)
            for sb, dr in [(a12, t_a12), (bhn, t_bhn), (ind, t_ind),
                           (wout, t_wout), (bout, t_bout)]:
                nc.sync.dma_start(sb[:], dr[:])
            h = hpool.tile([128, NL, B], bf16)
            nc.any.memset(h[:], 0.0)

            # staging for diffusion hops, baseline layout [(w%8)*16+f, cc, nl, b]
            xc1 = spool.tile([128, 3, NL, B], bf16)
            xc2 = spool.tile([128, 3, NL, B], bf16)

            # ---- phase 1: diffusion hops, chunked and DMA-pipelined ----
            with (
                tc.tile_pool(name="xmb", bufs=4) as xmpool,
                tc.tile_pool(name="p1", bufs=2, space="PSUM") as p1pool,
            ):
                wdmas = [(wc, t_wc, sl) for sl in range(8)] + \
                        [(whh, t_whh, sl) for sl in range(8)]
                for cc in range(3):
                    for bq in range(B // 4):
                        # trickle one GRU-weight slice DMA per chunk
                        if (cc * (B // 4) + bq) % 3 == 0 and wdmas:
                            sb, dr, sl = wdmas.pop(0)
                            nc.sync.dma_start(sb[:, 8 * sl:8 * sl + 8, :, :],
                                              dr[:, 8 * sl:8 * sl + 8, :, :])
                        xmb = xmpool.tile([128, 4, 4, 128], bf16, tag="xmb")
                        nc.sync.dma_start(xmb[:],
                                          t_xm[cc, :, 4 * bq:4 * bq + 4])
                        P12 = p1pool.tile([128, 2, NL, 4], f32, tag="P12")
                        for bs in range(4):
                            for mc in range(4):
                                nc.tensor.matmul(P12[:, :, :, bs],
                                                 xmb[:, bs, mc, :],
                                                 a12[:, mc, :, :],
                                                 start=(mc == 0),
                                                 stop=(mc == 3))
                        nc.vector.tensor_copy(
                            xc1[:, cc, :, 4 * bq:4 * bq + 4], P12[:, 0, :, :])
                        nc.scalar.copy(
                            xc2[:, cc, :, 4 * bq:4 * bq + 4], P12[:, 1, :, :])

            # ---- phase 2: GRU over time, input projection fused into Wc ----
            with (
                tc.tile_pool(name="xs", bufs=2) as xspool,
                tc.tile_pool(name="gp", bufs=2, space="PSUM") as gppool,
                tc.tile_pool(name="pn", bufs=2, space="PSUM") as pnpool,
                tc.tile_pool(name="hn", bufs=2, space="PSUM") as hnpool,
                tc.tile_pool(name="ew", bufs=3) as ewpool,
            ):
                for t in range(NT):
                    xs = xspool.tile([128, 2, NL, B], bf16, tag="xs")
                    # hop0 rows 0-15 + ones row 16 straight from DRAM
                    nc.sync.dma_start(xs[0:17, :, :, :], t_x0b[t])
                    # hop1/hop2 rows via SBUF->SBUF partition restructure
                    for wsub in range(2):
                        w = 2 * t + wsub
                        wo, ccw = w % 8, w // 8
                        src1 = xc1[16 * wo:16 * wo + 16, ccw, :, :]
                        src2 = xc2[16 * wo:16 * wo + 16, ccw, :, :]
                        nc.sync.dma_start(xs[17:33, wsub, :, :], src1)
                        nc.sync.dma_start(xs[33:49, wsub, :, :], src2)
                    if t < 2:
                        # rows 49-127 are contracted against zero weight rows;
                        # zero them once per ring buffer so no NaNs flow
                        nc.sync.dma_start(xs[KC:128, :, :, :], t_zpad[:])

                    for wsub in range(2):
                        for g in range(8):
                            ns = slice(8 * g, 8 * g + 8)
                            Prz = gppool.tile([128, 2, 8, B], f32, tag="Prz")
                            Pn = pnpool.tile([128, 8, B], f32, tag="Pn")
                            Phn = hnpool.tile([128, 8, B], f32, tag="Phn")
                            # b_hn via indicator matmul (opens accumulation)
                            nc.tensor.matmul(Phn[:], bhn[:, g, :], ind[:],
                                             start=True, stop=False,
                                             skip_group_check=True)
                            for j in range(8):
                                n = 8 * g + j
                                xsn = xs[:, wsub, n, :]
                                hn_ = h[:, n, :]
                                for gc in range(2):
                                    o = Prz[:, gc, j, :]
                                    nc.tensor.matmul(o, wc[:, n, gc, :], xsn,
                                                     start=True, stop=False)
                                    nc.tensor.matmul(o, whh[:, n, gc, :], hn_,
                                                     start=False, stop=True)
                                nc.tensor.matmul(Pn[:, j, :], wc[:, n, 2, :],
                                                 xsn, start=True, stop=True)
                                nc.tensor.matmul(Phn[:, j, :], whh[:, n, 2, :],
                                                 hn_, start=False,
                                                 stop=(j == 7),
                                                 skip_group_check=True)
                            # ---- gate elementwise (flat APs for 2x DVE) ----
                            hsl = h[:, ns, :].rearrange("p a b -> p (a b)")
                            rz = ewpool.tile([128, 2, 8, B], bf16, tag="rz")
                            r_ = rz[:, 0].rearrange("p a b -> p (a b)")
                            z_ = rz[:, 1].rearrange("p a b -> p (a b)")
                            nc.scalar.activation(
                                r_, Prz[:, 0].rearrange("p a b -> p (a b)"),
                                Act.Sigmoid)
                            nc.scalar.activation(
                                z_, Prz[:, 1].rearrange("p a b -> p (a b)"),
                                Act.Sigmoid)
                            tt = ewpool.tile([128, 8 * B], bf16, tag="tt")
                            nc.vector.tensor_tensor(
                                tt[:], r_,
                                Phn[:].rearrange("p a b -> p (a b)"), Alu.mult)
                            nc.vector.tensor_tensor(
                                tt[:], tt[:],
                                Pn[:].rearrange("p a b -> p (a b)"), Alu.add)
                            nt = ewpool.tile([128, 8 * B], bf16, tag="nt")
                            nc.scalar.activation(nt[:], tt[:], Act.Tanh)
                            st = ewpool.tile([128, 8 * B], bf16, tag="st")
                            nc.gpsimd.tensor_tensor(st[:], hsl, nt[:],
                                                    Alu.subtract)
                            nc.vector.tensor_tensor(st[:], z_, st[:], Alu.mult)
                            nc.vector.tensor_tensor(hsl, nt[:], st[:], Alu.add)

            # ---- output projection ----
            with (
                tc.tile_pool(name="po", bufs=1, space="PSUM") as popool,
                tc.tile_pool(name="ou", bufs=1) as oupool,
            ):
                Po = popool.tile([128, 32, F], f32)
                for c in range(32):
                    nc.tensor.matmul(Po[:, c, :], h[:, 2 * c:2 * c + 2, :],
                                     wout[:], start=True, stop=True)
                outsb = oupool.tile([128, 32, F], f32)
                nc.vector.tensor_tensor(
                    outsb[:], Po[:], bout[:, None, :].to_broadcast((128, 32, F)),
                    Alu.add)
                nc.sync.dma_start(t_out[:], outsb[:])
    nc.compile()


def kernel(**inputs):
    import concourse.bacc as bacc
    import concourse.bass as bass
    import concourse.mybir as mybir
    import concourse.tile as tile
    from concourse import bass_utils

    x = np.asarray(inputs["x"], np.float32)
    A = np.asarray(inputs["A_fw"], np.float32)
    dcw = np.asarray(inputs["dc_weights"], np.float32)
    W_ih = np.asarray(inputs["W_ih"], np.float32)
    W_hh = np.asarray(inputs["W_hh"], np.float32)
    b_ih = np.asarray(inputs["b_ih"], np.float32)
    b_hh = np.asarray(inputs["b_hh"], np.float32)
    W_out = np.asarray(inputs["W_out"], np.float32)
    b_out = np.asarray(inputs["b_out"], np.float32)

    A2 = A @ A
    dc_all = np.stack([dcw[0:16], dcw[16:32] + dcw[32:48], dcw[48:64] + dcw[64:80]])
    xbf = x.astype(BF)
    xm = np.ascontiguousarray(
        xbf.reshape(B, 4, 128, 3, 128).transpose(3, 2, 0, 1, 4))
    wout_h = W_out.astype(BF)
    bout_h = np.tile(b_out[None, :], (128, 1)).astype(np.float32)

    # fused input weights: Wc[n, gate, j, (hop, f)] = sum_h W_ih[n, gj, h] dc[hop, f, h]
    Wg = W_ih.reshape(N, 3, 128, H)
    wc_full = np.einsum('ngjh,ofh->ngjof', Wg, dc_all)        # [N, 3, 128, 3, 16]
    bias_in = b_ih.reshape(N, 3, 128).copy()
    bias_in[:, 0:2, :] += b_hh.reshape(N, 3, 128)[:, 0:2, :]  # r,z combined bias
    # K-layout: rows 0-15 hop0, 16 ones(bias), 17-32 hop1, 33-48 hop2,
    # 49-127 zero padding (keeps K=128 so fast weight load stays enabled)
    wc_k = np.zeros((N, 3, 128, 128), np.float32)
    wc_k[:, :, 0:16, :] = wc_full[:, :, :, 0, :].transpose(0, 1, 3, 2)
    wc_k[:, :, 16, :] = bias_in
    wc_k[:, :, 17:33, :] = wc_full[:, :, :, 1, :].transpose(0, 1, 3, 2)
    wc_k[:, :, 33:49, :] = wc_full[:, :, :, 2, :].transpose(0, 1, 3, 2)

    ind_h = np.zeros((128, 8, B), np.float32)
    for k in range(8):
        ind_h[k, k, :] = 1.0
    ind_h = ind_h.astype(BF)

    in_maps = []
    for c in range(8):
        ns = slice(c * NL, (c + 1) * NL)
        a1t = A[ns].T.astype(BF).reshape(4, 128, NL).transpose(1, 0, 2)
        a12 = np.ascontiguousarray(np.stack([a1t, A2[ns].T.astype(BF)
                                             .reshape(4, 128, NL)
                                             .transpose(1, 0, 2)], axis=2))
        xl = xbf[:, ns]  # [b, nl, w, f]
        # x0b[t, p, wsub, nl, b]: p 0-15 = f rows of x[w=2t+wsub], p16 = ones
        x0b = np.empty((NT, 17, 2, NL, B), np.float32)
        xw = np.asarray(xl, np.float32).transpose(2, 3, 1, 0)  # [w, f, nl, b]
        x0b[:, 0:16] = xw.reshape(NT, 2, 16, NL, B).transpose(0, 2, 1, 3, 4)
        x0b[:, 16] = 1.0
        wc_h = np.ascontiguousarray(
            wc_k[ns].transpose(2, 0, 1, 3)).astype(BF)       # [128, NL, 3, 128]
        whh_h = np.ascontiguousarray(
            W_hh[ns].transpose(2, 0, 1).astype(BF).reshape(128, NL, 3, 128))
        bhn_h = np.zeros((128, 8, 128), np.float32)           # [n8(+pad), g, j]
        bhn_h[0:8] = b_hh[ns, 256:384].reshape(8, 8, 128).transpose(1, 0, 2)
        bhn_h = bhn_h.astype(BF)
        in_maps.append({
            "a12": a12, "xm": xm,
            "x0b": np.ascontiguousarray(x0b).astype(BF),
            "wc": wc_h, "whh": whh_h, "bhn": bhn_h, "ind": ind_h,
            "zpad": np.zeros((128 - KC, 2, NL, B), BF),
            "wout": wout_h, "bout": bout_h,
        })

    nc = bacc.Bacc("TRN2", target_bir_lowering=False, debug=False, num_devices=8)
    _build(nc, bass, mybir, tile)
    import os, time
    trace = bool(os.environ.get("DGCN_TRACE"))
    res = bass_utils.run_bass_kernel_spmd(nc, in_maps, core_ids=list(range(8)),
                                          trace=trace)
    if trace and res.exec_time_ns:
        print(f"MEASURED exec_time_ns: {res.exec_time_ns}", flush=True)
        try:
            with open("/tmp/dgcn_exec_ns.txt", "w") as f:
                f.write(str(res.exec_time_ns))
        except Exception:
            pass
        if res.instructions_and_trace:
            print(f"trace: {res.instructions_and_trace[1]}", flush=True)
    if os.environ.get("DGCN_BENCH"):
        for it in range(int(os.environ["DGCN_BENCH"])):
            t0 = time.time()
            res = bass_utils.run_bass_kernel_spmd(nc, in_maps, core_ids=list(range(8)))
            print(f"bench iter {it}: {(time.time()-t0)*1e3:.1f} ms", flush=True)

    out = np.zeros((B, N, F), np.float32)
    for c in range(8):
        arr = res.results[c]["out"]  # [128, 32, F]
        tmp = arr.transpose(1, 0, 2).reshape(32, 2, B, F).transpose(2, 0, 1, 3)
        out[:, c * NL:(c + 1) * NL] = tmp.reshape(B, NL, F)
    return out
